# revision 18
# baseline (speedup 1.0000x reference)
"""Trainium2 Bass kernel for nn_NormalComparisonModel (dense comparison MLP).

Model: p1 = mean_L(f1), p2 = mean_L(f2);
out[i,j] = sigmoid(gelu(gelu([p1_i, p2_j, p1_i-p2_j] @ W1 + b1) @ W2 + b2) @ W3 + b3)

Key algebraic restructure: with U = W1[0:D] + W1[2D:3D], V = W1[D:2D] - W1[2D:3D],
    [p1_i, p2_j, p1_i-p2_j] @ W1 = p1_i @ U + p2_j @ V
so layer 1 collapses from O(N1*N2*3D*DENSE) to O((N1+N2)*D*DENSE) flops.

Sharding: data-parallel over N1 (32 rows/core); f2 pooling sharded over N2 with an
AllGather of the pooled features (32KB/core) so no core reads more than its own
4MB slice of either input.

Layouts: A2T / x1pre / x1T are j-major ([p, j*KCH+kc]) and B1T is i-major
([p, i*KCH+kc]) so the per-i bias broadcast has a contiguous last AP dim --
that qualifies the DVE tensor_tensor add for the 2x bf16 perf mode.
"""
import numpy as np

N_CORES = 8
N1, N2, L, D = 256, 256, 128, 256
IPC = N1 // N_CORES   # i rows per core
JPC = N2 // N_CORES   # j rows per core (pooling shard)
DENSE = 1024
H2 = 512
KCH = DENSE // 128    # 8 k-chunks
CB = H2 // 128        # 4 c-blocks
DC = D // 128         # 2 d-chunks

_CACHED_NC = None


def build_kernel():
    import concourse.bacc as bacc
    import concourse.mybir as mybir
    import concourse.tile as tile
    from concourse.ap import AP as APcls

    f32 = mybir.dt.float32
    f32r = mybir.dt.float32r
    bf16 = mybir.dt.bfloat16
    AF = mybir.ActivationFunctionType

    def bcast_at(ap_in, n, pos):
        """Insert a step-0 dim of size n at position pos of an AP."""
        lay = [list(d) for d in ap_in.ap]
        lay.insert(pos, [0, n])
        return APcls(ap_in.tensor, ap_in.offset, lay)

    nc = bacc.Bacc("TRN2", target_bir_lowering=False, debug=False,
                   num_devices=N_CORES)

    f1c = nc.declare_dram_parameter("f1c", [IPC, L, D], f32, isOutput=False)
    f2c = nc.declare_dram_parameter("f2c", [JPC, L, D], f32, isOutput=False)
    W1 = nc.declare_dram_parameter("W1", [3 * D, DENSE], f32, isOutput=False)
    b1 = nc.declare_dram_parameter("b1", [DENSE], f32, isOutput=False)
    W2 = nc.declare_dram_parameter("W2", [DENSE, H2], f32, isOutput=False)
    b2 = nc.declare_dram_parameter("b2", [H2], f32, isOutput=False)
    W3 = nc.declare_dram_parameter("W3", [H2, 1], f32, isOutput=False)
    b3r = nc.declare_dram_parameter("b3r", [IPC, 1], f32, isOutput=False)
    out_c = nc.declare_dram_parameter("out_c", [IPC, N2], f32, isOutput=True)

    with tile.TileContext(nc) as tc:
        with (
            tc.tile_pool(name="const", bufs=1) as cpool,
            tc.tile_pool(name="work", bufs=3) as wpool,
            tc.tile_pool(name="psum", bufs=2, space="PSUM") as pp,
            tc.tile_pool(name="dram", bufs=1, space="DRAM") as dpool,
        ):
            # ---------- weight prep first (DVE is in-order: U/V adds must not
            # queue behind the pooling copies) ----------
            ones_f32 = cpool.tile([128, 2], f32, tag="ones_f32")
            nc.vector.memset(ones_f32[:], 1.0 / L)
            ones_t = cpool.tile([128, 2], f32r, tag="ones")
            nc.vector.tensor_copy(ones_t[:], ones_f32[:])

            w1t = []
            for r in range(6):
                t = cpool.tile([128, DENSE], f32, tag=f"w1_{r}")
                nc.sync.dma_start(out=t[:], in_=W1[r * 128:(r + 1) * 128, :])
                w1t.append(t)
            Ut, Vt = [], []
            for dc in range(DC):
                u = cpool.tile([128, DENSE], f32r, tag=f"u_{dc}")
                v = cpool.tile([128, DENSE], f32r, tag=f"v_{dc}")
                nc.vector.tensor_add(u[:], w1t[dc][:], w1t[4 + dc][:])
                nc.vector.tensor_sub(v[:], w1t[2 + dc][:], w1t[4 + dc][:])
                Ut.append(u)
                Vt.append(v)

            # pT[d, row] = mean_l f[row, l, d]; f32r matmuls (f tile is the
            # stationary operand) accumulate columns into persistent psum tiles
            NLD = 4  # load chunks per source
            pool_sb = {}
            for (name, src, nrow) in (("p2", f2c, JPC), ("p1", f1c, IPC)):
                fall = cpool.tile([128, nrow * D], f32r, tag=f"fall_{name}")
                rows_per = nrow // NLD
                for c in range(NLD):
                    nc.gpsimd.dma_start(
                        out=fall[:, c * rows_per * D:(c + 1) * rows_per * D]
                            .rearrange("l (r d) -> l r d", r=rows_per),
                        in_=src[c * rows_per:(c + 1) * rows_per]
                            .rearrange("r l d -> l r d"))
                ps_dc = [pp.tile([128, 2 * nrow], f32, tag="small",
                                 name=f"ps_{name}_{dc}")
                         for dc in range(DC)]
                for r in range(nrow):
                    for dc in range(DC):
                        nc.tensor.matmul(
                            ps_dc[dc][:, 2 * r:2 * r + 2],
                            fall[:, r * D + dc * 128:r * D + (dc + 1) * 128],
                            ones_t[:], start=True, stop=True)
                pT = cpool.tile([128, DC * nrow], f32r, tag=f"{name}T")
                for dc in range(DC):
                    nc.vector.tensor_copy(pT[:, dc * nrow:(dc + 1) * nrow],
                                          ps_dc[dc][:, 0::2])
                pool_sb[name] = pT
            p2T = pool_sb["p2"]
            p1Tr = pool_sb["p1"]

            # ---------- AllGather p2T ----------
            p2loc = dpool.tile([DC, 128, JPC], f32, tag="p2loc")
            p2glob = dpool.tile([N_CORES, DC, 128, JPC], f32, tag="p2glob")
            nc.sync.dma_start(
                out=p2loc[:].rearrange("dc d j -> d dc j"),
                in_=p2T[:].bitcast(f32).rearrange("d (dc j) -> d dc j", dc=DC))
            nc.gpsimd.collective_compute(
                "AllGather",
                mybir.AluOpType.bypass,
                ins=[p2loc[:].opt()],
                outs=[p2glob[:].opt()],
                replica_groups=[list(range(N_CORES))],
            )
            p2all = []
            for dc in range(DC):
                t = cpool.tile([128, N2], f32r, tag=f"p2all_{dc}")
                nc.gpsimd.dma_start(
                    out=t[:].rearrange("d (c j) -> d c j", c=N_CORES),
                    in_=p2glob[:, dc, :, :].rearrange("c d j -> d c j"))
                p2all.append(t)

            # ---------- constants / weights ----------
            b1T = cpool.tile([128, KCH], f32, tag="b1T")     # b1T[p, kc] = b1[kc*128+p]
            nc.sync.dma_start(out=b1T[:], in_=b1[:].rearrange("(c p) -> p c", p=128))
            b2T = cpool.tile([128, CB], f32, tag="b2T")
            nc.sync.dma_start(out=b2T[:], in_=b2[:].rearrange("(c p) -> p c", p=128))
            w3T = cpool.tile([128, CB], bf16, tag="w3T")
            nc.gpsimd.dma_start(out=w3T[:],
                                in_=W3[:].rearrange("(c p) o -> p (c o)", p=128))
            b3t = cpool.tile([IPC, 1], f32, tag="b3t")
            nc.sync.dma_start(out=b3t[:], in_=b3r[:])

            # W2 in bf16, one mega cast DMA; w2m[:, kc*H2 + c]
            w2m = cpool.tile([128, KCH * H2], bf16, tag="w2m")
            nc.gpsimd.dma_start(
                out=w2m[:].rearrange("p (kc c) -> p kc c", kc=KCH),
                in_=W2[:].rearrange("(kc p) c -> p kc c", p=128))

            # ---------- A2T first (j-gathered side gates the loop), then B1T ----
            B1T = cpool.tile([128, IPC * KCH], bf16, tag="B1T")  # col = i*KCH + kc
            A2T = cpool.tile([128, KCH * N2], bf16, tag="A2T")   # col = kc*N2 + j
            for kb in range(KCH):
                pa2 = pp.tile([128, N2], f32, tag="big")
                for dc in range(DC):
                    nc.tensor.matmul(pa2[:], Vt[dc][:, kb * 128:(kb + 1) * 128],
                                     p2all[dc][:], start=(dc == 0),
                                     stop=(dc == DC - 1))
                nc.vector.tensor_copy(A2T[:, kb * N2:(kb + 1) * N2], pa2[:])
            for kb in range(KCH):
                pa = pp.tile([128, IPC], f32, tag="small")
                for dc in range(DC):
                    nc.tensor.matmul(pa[:], Ut[dc][:, kb * 128:(kb + 1) * 128],
                                     p1Tr[:, dc * IPC:(dc + 1) * IPC],
                                     start=(dc == 0), stop=(dc == DC - 1))
                nc.vector.tensor_scalar_add(B1T[:, kb::KCH], pa[:],
                                            b1T[:, kb:kb + 1])

            # ---------- main loop over i-pairs ----------
            # Pair layout: x1 cols = kc*(2*N2) + ip*N2 + j  (kc-major, ip inner)
            NJ2 = 2 * N2
            outst = cpool.tile([IPC, N2], f32, tag="outst")
            for g in range(IPC // 2):
                # x1pre[p, kc, ip, j] = A2T[p, kc, j] + B1T[p, (2g+ip)*KCH+kc]
                x1pre = wpool.tile([128, KCH * NJ2], bf16, tag="x1pre")
                nc.vector.tensor_add(
                    x1pre[:].rearrange("p (k ip j) -> p k ip j", k=KCH, ip=2),
                    bcast_at(A2T[:].rearrange("p (k j) -> p k j", k=KCH), 2, 2),
                    B1T[:, 2 * g * KCH:(2 * g + 2) * KCH]
                        .rearrange("p (ip k) -> p k ip", ip=2)
                        .broadcast_to((128, KCH, 2, N2)))
                # x1 = gelu(x1pre) -> bf16 (one op for the pair)
                x1T = wpool.tile([128, KCH * NJ2], bf16, tag="x1T")
                nc.scalar.activation(x1T[:], x1pre[:], AF.Gelu)

                # layer 2 in two cb-halves; h2 = gelu(. + b2) on ACT (bias is
                # per-partition within each cb slice)
                h2T = []
                for cbh in range(2):
                    ph2 = pp.tile([128, 2 * NJ2], f32, tag="big",
                                  name=f"ph2_{g}_{cbh}")
                    for cbi in range(2):
                        cb = 2 * cbh + cbi
                        for kc in range(KCH):
                            nc.tensor.matmul(
                                ph2[:, cbi * NJ2:(cbi + 1) * NJ2],
                                w2m[:, kc * H2 + cb * 128:kc * H2 + (cb + 1) * 128],
                                x1T[:, kc * NJ2:(kc + 1) * NJ2],
                                start=(kc == 0), stop=(kc == KCH - 1))
                    ht = wpool.tile([128, 2 * NJ2], bf16, tag="h2T",
                                    name=f"h2T_{g}_{cbh}")
                    for cbi in range(2):
                        cb = 2 * cbh + cbi
                        nc.scalar.activation(ht[:, cbi * NJ2:(cbi + 1) * NJ2],
                                             ph2[:, cbi * NJ2:(cbi + 1) * NJ2],
                                             AF.Gelu, bias=b2T[:, cb:cb + 1])
                    h2T.append(ht)

                # layer 3: out_pre[1, ip*N2+j] = sum_cb w3T[:, cb].T @ h2T[cb]
                pl3 = pp.tile([1, NJ2], f32, tag="small")
                for cb in range(CB):
                    nc.tensor.matmul(
                        pl3[:], w3T[:, cb:cb + 1],
                        h2T[cbh := cb // 2][:, (cb % 2) * NJ2:(cb % 2 + 1) * NJ2],
                        start=(cb == 0), stop=(cb == CB - 1))
                # collect rows 2g, 2g+1 (cross-partition move via small DMA)
                orow = wpool.tile([1, NJ2], f32, tag="orow")
                nc.vector.tensor_copy(orow[:], pl3[:])
                nc.sync.dma_start(out=outst[2 * g:2 * g + 1, :],
                                  in_=orow[0:1, 0:N2])
                nc.sync.dma_start(out=outst[2 * g + 1:2 * g + 2, :],
                                  in_=orow[0:1, N2:NJ2])

            # ---------- sigmoid + store ----------
            osg = cpool.tile([IPC, N2], f32, tag="osg")
            nc.scalar.activation(osg[:], outst[:], AF.Sigmoid, bias=b3t[:])
            nc.sync.dma_start(out=out_c[:], in_=osg[:])

    nc.finalize()
    return nc


def kernel(**inputs):
    from concourse.bass_utils import run_bass_kernel_spmd

    global _CACHED_NC
    f1 = np.ascontiguousarray(np.asarray(inputs["f1"], dtype=np.float32))
    f2 = np.ascontiguousarray(np.asarray(inputs["f2"], dtype=np.float32))
    W1 = np.ascontiguousarray(np.asarray(inputs["W1"], dtype=np.float32))
    b1 = np.asarray(inputs["b1"], dtype=np.float32)
    W2 = np.ascontiguousarray(np.asarray(inputs["W2"], dtype=np.float32))
    b2 = np.asarray(inputs["b2"], dtype=np.float32)
    W3 = np.ascontiguousarray(np.asarray(inputs["W3"], dtype=np.float32))
    b3 = np.asarray(inputs["b3"], dtype=np.float32)
    b3r = np.full((IPC, 1), b3.reshape(-1)[0], dtype=np.float32)

    if _CACHED_NC is None:
        _CACHED_NC = build_kernel()
    nc = _CACHED_NC

    in_maps = []
    for k in range(N_CORES):
        in_maps.append({
            "f1c": np.ascontiguousarray(f1[k * IPC:(k + 1) * IPC]),
            "f2c": np.ascontiguousarray(f2[k * JPC:(k + 1) * JPC]),
            "W1": W1, "b1": b1, "W2": W2, "b2": b2, "W3": W3, "b3r": b3r,
        })
    res = run_bass_kernel_spmd(nc, in_maps, core_ids=list(range(N_CORES)))
    out = np.concatenate([res.results[k]["out_c"] for k in range(N_CORES)],
                         axis=0)
    return out.astype(np.float32)


# revision 25
# speedup vs baseline: 1.5142x; 1.5142x over previous
"""Trainium2 Bass kernel for nn_NormalComparisonModel (dense comparison MLP).

Model: p1 = mean_L(f1), p2 = mean_L(f2);
out[i,j] = sigmoid(gelu(gelu([p1_i, p2_j, p1_i-p2_j] @ W1 + b1) @ W2 + b2) @ W3 + b3)

Key algebraic restructure: with U = W1[0:D] + W1[2D:3D], V = W1[D:2D] - W1[2D:3D],
    [p1_i, p2_j, p1_i-p2_j] @ W1 = p1_i @ U + p2_j @ V
so layer 1 collapses from O(N1*N2*3D*DENSE) to O((N1+N2)*D*DENSE) flops.

Sharding: data-parallel over N1 (32 rows/core); f2 pooling sharded over N2 with an
AllGather of the pooled features (32KB/core) so no core reads more than its own
4MB slice of either input.

Layouts: A2T / x1pre / x1T are j-major ([p, j*KCH+kc]) and B1T is i-major
([p, i*KCH+kc]) so the per-i bias broadcast has a contiguous last AP dim --
that qualifies the DVE tensor_tensor add for the 2x bf16 perf mode.
"""
import numpy as np

N_CORES = 8
N1, N2, L, D = 256, 256, 128, 256
IPC = N1 // N_CORES   # i rows per core
JPC = N2 // N_CORES   # j rows per core (pooling shard)
DENSE = 1024
H2 = 512
KCH = DENSE // 128    # 8 k-chunks
CB = H2 // 128        # 4 c-blocks
DC = D // 128         # 2 d-chunks

_CACHED_NC = None


def build_kernel():
    import concourse.bacc as bacc
    import concourse.mybir as mybir
    import concourse.tile as tile
    from concourse.ap import AP as APcls

    f32 = mybir.dt.float32
    f32r = mybir.dt.float32r
    bf16 = mybir.dt.bfloat16
    AF = mybir.ActivationFunctionType

    def bcast_at(ap_in, n, pos):
        """Insert a step-0 dim of size n at position pos of an AP."""
        lay = [list(d) for d in ap_in.ap]
        lay.insert(pos, [0, n])
        return APcls(ap_in.tensor, ap_in.offset, lay)

    nc = bacc.Bacc("TRN2", target_bir_lowering=False, debug=False,
                   num_devices=N_CORES)

    f1c = nc.declare_dram_parameter("f1c", [IPC, L, D], f32, isOutput=False)
    f2c = nc.declare_dram_parameter("f2c", [JPC, L, D], f32, isOutput=False)
    W1 = nc.declare_dram_parameter("W1", [3 * D, DENSE], f32, isOutput=False)
    b1 = nc.declare_dram_parameter("b1", [DENSE], f32, isOutput=False)
    W2 = nc.declare_dram_parameter("W2", [DENSE, H2], f32, isOutput=False)
    b2 = nc.declare_dram_parameter("b2", [H2], f32, isOutput=False)
    W3 = nc.declare_dram_parameter("W3", [H2, 1], f32, isOutput=False)
    b3r = nc.declare_dram_parameter("b3r", [IPC, 1], f32, isOutput=False)
    out_c = nc.declare_dram_parameter("out_c", [IPC, N2], f32, isOutput=True)

    with tile.TileContext(nc) as tc:
        with (
            tc.tile_pool(name="const", bufs=1) as cpool,
            tc.tile_pool(name="work", bufs=3) as wpool,
            tc.tile_pool(name="psum", bufs=2, space="PSUM") as pp,
            tc.tile_pool(name="dram", bufs=1, space="DRAM") as dpool,
        ):
            # ---------- weight prep first (DVE is in-order: U/V adds must not
            # queue behind the pooling copies) ----------
            ones_f32 = cpool.tile([128, 2], f32, tag="ones_f32")
            nc.vector.memset(ones_f32[:], 1.0 / L)
            ones_t = cpool.tile([128, 2], f32r, tag="ones")
            nc.vector.tensor_copy(ones_t[:], ones_f32[:])

            w1t = []
            for r in range(6):
                t = cpool.tile([128, DENSE], f32, tag=f"w1_{r}")
                nc.sync.dma_start(out=t[:], in_=W1[r * 128:(r + 1) * 128, :])
                w1t.append(t)
            Ut, Vt = [], []
            for dc in range(DC):
                u = cpool.tile([128, DENSE], f32r, tag=f"u_{dc}")
                v = cpool.tile([128, DENSE], f32r, tag=f"v_{dc}")
                nc.vector.tensor_add(u[:], w1t[dc][:], w1t[4 + dc][:])
                nc.vector.tensor_sub(v[:], w1t[2 + dc][:], w1t[4 + dc][:])
                Ut.append(u)
                Vt.append(v)

            # pT[d, row] = mean_l f[row, l, d]; f32r matmuls (f tile is the
            # stationary operand) accumulate columns into persistent psum tiles
            NLD = 4  # load chunks per source

            def pool_source(name, src, nrow):
                fall = cpool.tile([128, nrow * D], f32r, tag=f"fall_{name}",
                                  name=f"fall_{name}")
                rows_per = nrow // NLD
                for c in range(NLD):
                    nc.gpsimd.dma_start(
                        out=fall[:, c * rows_per * D:(c + 1) * rows_per * D]
                            .rearrange("l (r d) -> l r d", r=rows_per),
                        in_=src[c * rows_per:(c + 1) * rows_per]
                            .rearrange("r l d -> l r d"))
                ps_dc = [pp.tile([128, 2 * nrow], f32, tag="small",
                                 name=f"ps_{name}_{dc}")
                         for dc in range(DC)]
                for r in range(nrow):
                    for dc in range(DC):
                        nc.tensor.matmul(
                            ps_dc[dc][:, 2 * r:2 * r + 2],
                            fall[:, r * D + dc * 128:r * D + (dc + 1) * 128],
                            ones_t[:], start=True, stop=True)
                pT = cpool.tile([128, DC * nrow], f32r, tag=f"{name}T",
                                name=f"{name}T")
                for dc in range(DC):
                    nc.vector.tensor_copy(pT[:, dc * nrow:(dc + 1) * nrow],
                                          ps_dc[dc][:, 0::2])
                return pT

            p2T = pool_source("p2", f2c, JPC)

            # ---------- local A2T chunk (pre-AllGather): A2Tl = V.T @ p2T_local --
            A2Tl = cpool.tile([128, KCH * JPC], bf16, tag="A2Tl")
            for kb in range(KCH):
                pal = pp.tile([128, JPC], f32, tag="small", name=f"pal_{kb}")
                for dc in range(DC):
                    nc.tensor.matmul(pal[:], Vt[dc][:, kb * 128:(kb + 1) * 128],
                                     p2T[:, dc * JPC:(dc + 1) * JPC],
                                     start=(dc == 0), stop=(dc == DC - 1))
                nc.vector.tensor_copy(A2Tl[:, kb * JPC:(kb + 1) * JPC], pal[:])

            # ---------- AllGather A2T (bf16, 64KB per core) ----------
            agloc = dpool.tile([KCH, 128, JPC], bf16, tag="agloc")
            agglob = dpool.tile([N_CORES, KCH, 128, JPC], bf16, tag="agglob")
            nc.sync.dma_start(
                out=agloc[:].rearrange("k d j -> d k j"),
                in_=A2Tl[:].rearrange("d (k j) -> d k j", k=KCH))
            nc.gpsimd.collective_compute(
                "AllGather",
                mybir.AluOpType.bypass,
                ins=[agloc[:].opt()],
                outs=[agglob[:].opt()],
                replica_groups=[list(range(N_CORES))],
            )
            A2T = cpool.tile([128, KCH * N2], bf16, tag="A2T")   # col = kc*N2 + j
            for k in range(KCH):
                nc.sync.dma_start(
                    out=A2T[:, k * N2:(k + 1) * N2]
                        .rearrange("d (c j) -> d c j", c=N_CORES),
                    in_=agglob[:, k, :, :].rearrange("c d j -> d c j"))

            # ---------- f1 pooling (under the AllGather latency) ----------
            p1Tr = pool_source("p1", f1c, IPC)

            # ---------- constants / weights ----------
            b1T = cpool.tile([128, KCH], f32, tag="b1T")     # b1T[p, kc] = b1[kc*128+p]
            nc.sync.dma_start(out=b1T[:], in_=b1[:].rearrange("(c p) -> p c", p=128))
            b2T = cpool.tile([128, CB], f32, tag="b2T")
            nc.sync.dma_start(out=b2T[:], in_=b2[:].rearrange("(c p) -> p c", p=128))
            w3T = cpool.tile([128, CB], bf16, tag="w3T")
            nc.gpsimd.dma_start(out=w3T[:],
                                in_=W3[:].rearrange("(c p) o -> p (c o)", p=128))
            b3t = cpool.tile([IPC, 1], f32, tag="b3t")
            nc.sync.dma_start(out=b3t[:], in_=b3r[:])

            # W2 in bf16, one mega cast DMA; w2m[:, kc*H2 + c]
            w2m = cpool.tile([128, KCH * H2], bf16, tag="w2m")
            nc.gpsimd.dma_start(
                out=w2m[:].rearrange("p (kc c) -> p kc c", kc=KCH),
                in_=W2[:].rearrange("(kc p) c -> p kc c", p=128))

            # ---------- B1T = U.T @ p1T + b1 (i-major) ----------
            B1T = cpool.tile([128, IPC * KCH], bf16, tag="B1T")  # col = i*KCH + kc
            for kb in range(KCH):
                pa = pp.tile([128, IPC], f32, tag="small")
                for dc in range(DC):
                    nc.tensor.matmul(pa[:], Ut[dc][:, kb * 128:(kb + 1) * 128],
                                     p1Tr[:, dc * IPC:(dc + 1) * IPC],
                                     start=(dc == 0), stop=(dc == DC - 1))
                nc.vector.tensor_scalar_add(B1T[:, kb::KCH], pa[:],
                                            b1T[:, kb:kb + 1])

            # ---------- main loop over i-pairs (software-pipelined) ----------
            # Pair layout: x1 cols = kc*(2*N2) + ip*N2 + j  (kc-major, ip inner)
            NJ2 = 2 * N2
            outst = cpool.tile([IPC, N2], f32, tag="outst")
            pl3_prev = None

            def emit_collect(g, pl3):
                orow = wpool.tile([1, NJ2], f32, tag="orow", name=f"orow_{g}")
                nc.vector.tensor_copy(orow[:], pl3[:])
                nc.sync.dma_start(out=outst[2 * g:2 * g + 1, :],
                                  in_=orow[0:1, 0:N2])
                nc.sync.dma_start(out=outst[2 * g + 1:2 * g + 2, :],
                                  in_=orow[0:1, N2:NJ2])

            for g in range(IPC // 2):
                # x1pre[p, kc, ip, j] = A2T[p, kc, j] + B1T[p, (2g+ip)*KCH+kc]
                x1pre = wpool.tile([128, KCH * NJ2], bf16, tag="x1pre")
                nc.vector.tensor_add(
                    x1pre[:].rearrange("p (k ip j) -> p k ip j", k=KCH, ip=2),
                    bcast_at(A2T[:].rearrange("p (k j) -> p k j", k=KCH), 2, 2),
                    B1T[:, 2 * g * KCH:(2 * g + 2) * KCH]
                        .rearrange("p (ip k) -> p k ip", ip=2)
                        .broadcast_to((128, KCH, 2, N2)))
                # x1 = gelu(x1pre) -> bf16 (one op for the pair)
                x1T = wpool.tile([128, KCH * NJ2], bf16, tag="x1T")
                nc.scalar.activation(x1T[:], x1pre[:], AF.Gelu)

                # previous pair's output collection (deps long since ready, so
                # it doesn't stall the in-order DVE/DMA queues)
                if pl3_prev is not None:
                    emit_collect(g - 1, pl3_prev)

                # layer 2, one psum tile per cb; h2 = gelu(. + b2) on ACT
                h2T = []
                for cb in range(CB):
                    ph2 = pp.tile([128, NJ2], f32, tag="ph2", bufs=4,
                                  name=f"ph2_{g}_{cb}")
                    for kc in range(KCH):
                        nc.tensor.matmul(
                            ph2[:],
                            w2m[:, kc * H2 + cb * 128:kc * H2 + (cb + 1) * 128],
                            x1T[:, kc * NJ2:(kc + 1) * NJ2],
                            start=(kc == 0), stop=(kc == KCH - 1))
                    ht = wpool.tile([128, NJ2], bf16, tag="h2T",
                                    name=f"h2T_{g}_{cb}", bufs=6)
                    nc.scalar.activation(ht[:], ph2[:], AF.Gelu,
                                         bias=b2T[:, cb:cb + 1])
                    h2T.append(ht)

                # layer 3: out_pre[1, ip*N2+j] = sum_cb w3T[:, cb].T @ h2T[cb]
                pl3 = pp.tile([1, NJ2], f32, tag="small")
                for cb in range(CB):
                    nc.tensor.matmul(pl3[:], w3T[:, cb:cb + 1], h2T[cb][:],
                                     start=(cb == 0), stop=(cb == CB - 1))
                pl3_prev = pl3
            emit_collect(IPC // 2 - 1, pl3_prev)

            # ---------- sigmoid + store ----------
            osg = cpool.tile([IPC, N2], f32, tag="osg")
            nc.scalar.activation(osg[:], outst[:], AF.Sigmoid, bias=b3t[:])
            nc.sync.dma_start(out=out_c[:], in_=osg[:])

    nc.finalize()
    return nc


def kernel(**inputs):
    from concourse.bass_utils import run_bass_kernel_spmd

    global _CACHED_NC
    f1 = np.ascontiguousarray(np.asarray(inputs["f1"], dtype=np.float32))
    f2 = np.ascontiguousarray(np.asarray(inputs["f2"], dtype=np.float32))
    W1 = np.ascontiguousarray(np.asarray(inputs["W1"], dtype=np.float32))
    b1 = np.asarray(inputs["b1"], dtype=np.float32)
    W2 = np.ascontiguousarray(np.asarray(inputs["W2"], dtype=np.float32))
    b2 = np.asarray(inputs["b2"], dtype=np.float32)
    W3 = np.ascontiguousarray(np.asarray(inputs["W3"], dtype=np.float32))
    b3 = np.asarray(inputs["b3"], dtype=np.float32)
    b3r = np.full((IPC, 1), b3.reshape(-1)[0], dtype=np.float32)

    if _CACHED_NC is None:
        _CACHED_NC = build_kernel()
    nc = _CACHED_NC

    in_maps = []
    for k in range(N_CORES):
        in_maps.append({
            "f1c": np.ascontiguousarray(f1[k * IPC:(k + 1) * IPC]),
            "f2c": np.ascontiguousarray(f2[k * JPC:(k + 1) * JPC]),
            "W1": W1, "b1": b1, "W2": W2, "b2": b2, "W3": W3, "b3r": b3r,
        })
    res = run_bass_kernel_spmd(nc, in_maps, core_ids=list(range(N_CORES)))
    out = np.concatenate([res.results[k]["out_c"] for k in range(N_CORES)],
                         axis=0)
    return out.astype(np.float32)


# revision 30
# speedup vs baseline: 1.5599x; 1.0302x over previous
"""Trainium2 Bass kernel for nn_NormalComparisonModel (dense comparison MLP).

Model: p1 = mean_L(f1), p2 = mean_L(f2);
out[i,j] = sigmoid(gelu(gelu([p1_i, p2_j, p1_i-p2_j] @ W1 + b1) @ W2 + b2) @ W3 + b3)

Key algebraic restructure: with U = W1[0:D] + W1[2D:3D], V = W1[D:2D] - W1[2D:3D],
    [p1_i, p2_j, p1_i-p2_j] @ W1 = p1_i @ U + p2_j @ V
so layer 1 collapses from O(N1*N2*3D*DENSE) to O((N1+N2)*D*DENSE) flops.

Sharding: data-parallel over N1 (32 rows/core); f2 pooling sharded over N2 with an
AllGather of the pooled features (32KB/core) so no core reads more than its own
4MB slice of either input.

Layouts: A2T / x1pre / x1T are j-major ([p, j*KCH+kc]) and B1T is i-major
([p, i*KCH+kc]) so the per-i bias broadcast has a contiguous last AP dim --
that qualifies the DVE tensor_tensor add for the 2x bf16 perf mode.
"""
import numpy as np

N_CORES = 8
N1, N2, L, D = 256, 256, 128, 256
IPC = N1 // N_CORES   # i rows per core
JPC = N2 // N_CORES   # j rows per core (pooling shard)
DENSE = 1024
H2 = 512
KCH = DENSE // 128    # 8 k-chunks
CB = H2 // 128        # 4 c-blocks
DC = D // 128         # 2 d-chunks

_CACHED_NC = None


def build_kernel():
    import concourse.bacc as bacc
    import concourse.mybir as mybir
    import concourse.tile as tile
    from concourse.ap import AP as APcls

    f32 = mybir.dt.float32
    f32r = mybir.dt.float32r
    bf16 = mybir.dt.bfloat16
    AF = mybir.ActivationFunctionType

    def bcast_at(ap_in, n, pos):
        """Insert a step-0 dim of size n at position pos of an AP."""
        lay = [list(d) for d in ap_in.ap]
        lay.insert(pos, [0, n])
        return APcls(ap_in.tensor, ap_in.offset, lay)

    nc = bacc.Bacc("TRN2", target_bir_lowering=False, debug=False,
                   num_devices=N_CORES)

    f1c = nc.declare_dram_parameter("f1c", [IPC, L, D], f32, isOutput=False)
    f2c = nc.declare_dram_parameter("f2c", [JPC, L, D], f32, isOutput=False)
    W1 = nc.declare_dram_parameter("W1", [3 * D, DENSE], f32, isOutput=False)
    b1 = nc.declare_dram_parameter("b1", [DENSE], f32, isOutput=False)
    W2 = nc.declare_dram_parameter("W2", [DENSE, H2], f32, isOutput=False)
    b2 = nc.declare_dram_parameter("b2", [H2], f32, isOutput=False)
    W3 = nc.declare_dram_parameter("W3", [H2, 1], f32, isOutput=False)
    b3r = nc.declare_dram_parameter("b3r", [IPC, 1], f32, isOutput=False)
    out_c = nc.declare_dram_parameter("out_c", [IPC, N2], f32, isOutput=True)

    with tile.TileContext(nc) as tc:
        with (
            tc.tile_pool(name="const", bufs=1) as cpool,
            tc.tile_pool(name="work", bufs=3) as wpool,
            tc.tile_pool(name="psum", bufs=2, space="PSUM") as pp,
            tc.tile_pool(name="dram", bufs=1, space="DRAM") as dpool,
        ):
            # ---------- weight prep first (DVE is in-order: U/V adds must not
            # queue behind the pooling copies) ----------
            ones_f32 = cpool.tile([128, 2], f32, tag="ones_f32")
            nc.vector.memset(ones_f32[:], 1.0 / L)
            ones_t = cpool.tile([128, 2], f32r, tag="ones")
            nc.vector.tensor_copy(ones_t[:], ones_f32[:])

            w1t = []
            for r in range(6):
                t = cpool.tile([128, DENSE], f32, tag=f"w1_{r}")
                nc.sync.dma_start(out=t[:], in_=W1[r * 128:(r + 1) * 128, :])
                w1t.append(t)
            Ut, Vt = [], []
            for dc in range(DC):
                u = cpool.tile([128, DENSE], f32r, tag=f"u_{dc}")
                v = cpool.tile([128, DENSE], f32r, tag=f"v_{dc}")
                nc.vector.tensor_add(u[:], w1t[dc][:], w1t[4 + dc][:])
                nc.vector.tensor_sub(v[:], w1t[2 + dc][:], w1t[4 + dc][:])
                Ut.append(u)
                Vt.append(v)

            # pT[d, row] = mean_l f[row, l, d]; f32r matmuls (f tile is the
            # stationary operand) accumulate columns into persistent psum tiles
            NLD = 4  # load chunks per source

            def pool_source(name, src, nrow):
                fall = cpool.tile([128, nrow * D], f32r, tag=f"fall_{name}",
                                  name=f"fall_{name}")
                rows_per = nrow // NLD
                for c in range(NLD):
                    nc.gpsimd.dma_start(
                        out=fall[:, c * rows_per * D:(c + 1) * rows_per * D]
                            .rearrange("l (r d) -> l r d", r=rows_per),
                        in_=src[c * rows_per:(c + 1) * rows_per]
                            .rearrange("r l d -> l r d"))
                ps_dc = [pp.tile([128, 2 * nrow], f32, tag="small",
                                 name=f"ps_{name}_{dc}")
                         for dc in range(DC)]
                for r in range(nrow):
                    for dc in range(DC):
                        nc.tensor.matmul(
                            ps_dc[dc][:, 2 * r:2 * r + 2],
                            fall[:, r * D + dc * 128:r * D + (dc + 1) * 128],
                            ones_t[:], start=True, stop=True)
                pT = cpool.tile([128, DC * nrow], f32r, tag=f"{name}T",
                                name=f"{name}T")
                for dc in range(DC):
                    nc.vector.tensor_copy(pT[:, dc * nrow:(dc + 1) * nrow],
                                          ps_dc[dc][:, 0::2])
                return pT

            p2T = pool_source("p2", f2c, JPC)

            # ---------- local A2T chunk (pre-AllGather): A2Tl = V.T @ p2T_local --
            A2Tl = cpool.tile([128, KCH * JPC], bf16, tag="A2Tl")
            for kb in range(KCH):
                pal = pp.tile([128, JPC], f32, tag="small", name=f"pal_{kb}")
                for dc in range(DC):
                    nc.tensor.matmul(pal[:], Vt[dc][:, kb * 128:(kb + 1) * 128],
                                     p2T[:, dc * JPC:(dc + 1) * JPC],
                                     start=(dc == 0), stop=(dc == DC - 1))
                nc.vector.tensor_copy(A2Tl[:, kb * JPC:(kb + 1) * JPC], pal[:])

            # ---------- AllGather A2T (bf16, 64KB per core) ----------
            agloc = dpool.tile([KCH, 128, JPC], bf16, tag="agloc")
            agglob = dpool.tile([N_CORES, KCH, 128, JPC], bf16, tag="agglob")
            nc.sync.dma_start(
                out=agloc[:].rearrange("k d j -> d k j"),
                in_=A2Tl[:].rearrange("d (k j) -> d k j", k=KCH))
            nc.gpsimd.collective_compute(
                "AllGather",
                mybir.AluOpType.bypass,
                ins=[agloc[:].opt()],
                outs=[agglob[:].opt()],
                replica_groups=[list(range(N_CORES))],
            )
            A2T = cpool.tile([128, KCH * N2], bf16, tag="A2T")   # col = kc*N2 + j
            for k in range(KCH):
                nc.sync.dma_start(
                    out=A2T[:, k * N2:(k + 1) * N2]
                        .rearrange("d (c j) -> d c j", c=N_CORES),
                    in_=agglob[:, k, :, :].rearrange("c d j -> d c j"))

            # ---------- f1 pooling (under the AllGather latency) ----------
            p1Tr = pool_source("p1", f1c, IPC)

            # ---------- constants / weights ----------
            b1T = cpool.tile([128, KCH], f32, tag="b1T")     # b1T[p, kc] = b1[kc*128+p]
            nc.sync.dma_start(out=b1T[:], in_=b1[:].rearrange("(c p) -> p c", p=128))
            b2T = cpool.tile([128, CB], f32, tag="b2T")
            nc.sync.dma_start(out=b2T[:], in_=b2[:].rearrange("(c p) -> p c", p=128))
            w3T = cpool.tile([128, CB], bf16, tag="w3T")
            nc.gpsimd.dma_start(out=w3T[:],
                                in_=W3[:].rearrange("(c p) o -> p (c o)", p=128))
            b3t = cpool.tile([IPC, 1], f32, tag="b3t")
            nc.sync.dma_start(out=b3t[:], in_=b3r[:])

            # W2 in bf16, one mega cast DMA; w2m[:, kc*H2 + c]
            w2m = cpool.tile([128, KCH * H2], bf16, tag="w2m")
            nc.gpsimd.dma_start(
                out=w2m[:].rearrange("p (kc c) -> p kc c", kc=KCH),
                in_=W2[:].rearrange("(kc p) c -> p kc c", p=128))

            # ---------- B1T = U.T @ p1T + b1 (i-major) ----------
            B1T = cpool.tile([128, IPC * KCH], bf16, tag="B1T")  # col = i*KCH + kc
            for kb in range(KCH):
                pa = pp.tile([128, IPC], f32, tag="small")
                for dc in range(DC):
                    nc.tensor.matmul(pa[:], Ut[dc][:, kb * 128:(kb + 1) * 128],
                                     p1Tr[:, dc * IPC:(dc + 1) * IPC],
                                     start=(dc == 0), stop=(dc == DC - 1))
                nc.vector.tensor_scalar_add(B1T[:, kb::KCH], pa[:],
                                            b1T[:, kb:kb + 1])

            # ---------- main loop over i-pairs (software-pipelined) ----------
            # Pair layout: x1 cols = kc*(2*N2) + ip*N2 + j  (kc-major, ip inner)
            NJ2 = 2 * N2
            outst = cpool.tile([IPC, N2], f32, tag="outst")
            pl3_prev = None
            h2T_prev = None

            def emit_collect(g, pl3):
                orow = wpool.tile([1, NJ2], f32, tag="orow", name=f"orow_{g}")
                nc.vector.tensor_copy(orow[:], pl3[:])
                nc.sync.dma_start(out=outst[2 * g:2 * g + 1, :],
                                  in_=orow[0:1, 0:N2])
                nc.sync.dma_start(out=outst[2 * g + 1:2 * g + 2, :],
                                  in_=orow[0:1, N2:NJ2])

            def emit_l3(g, h2T):
                pl3 = pp.tile([1, NJ2], f32, tag="small", name=f"pl3_{g}")
                for cb in range(CB):
                    nc.tensor.matmul(pl3[:], w3T[:, cb:cb + 1], h2T[cb][:],
                                     start=(cb == 0), stop=(cb == CB - 1))
                return pl3

            for g in range(IPC // 2):
                # x1pre[p, kc, ip, j] = A2T[p, kc, j] + B1T[p, (2g+ip)*KCH+kc]
                x1pre = wpool.tile([128, KCH * NJ2], bf16, tag="x1pre")
                nc.vector.tensor_add(
                    x1pre[:].rearrange("p (k ip j) -> p k ip j", k=KCH, ip=2),
                    bcast_at(A2T[:].rearrange("p (k j) -> p k j", k=KCH), 2, 2),
                    B1T[:, 2 * g * KCH:(2 * g + 2) * KCH]
                        .rearrange("p (ip k) -> p k ip", ip=2)
                        .broadcast_to((128, KCH, 2, N2)))
                # x1 = gelu(x1pre) -> bf16 (one op for the pair)
                x1T = wpool.tile([128, KCH * NJ2], bf16, tag="x1T")
                nc.scalar.activation(x1T[:], x1pre[:], AF.Gelu)

                # previous pair's layer 3 + output collection (deps long since
                # ready, so they don't stall the in-order PE/DVE/DMA queues)
                if h2T_prev is not None:
                    pl3_prev = emit_l3(g - 1, h2T_prev)
                    emit_collect(g - 1, pl3_prev)

                # layer 2, one psum tile per cb; h2 = gelu(. + b2) on ACT
                h2T = []
                for cb in range(CB):
                    ph2 = pp.tile([128, NJ2], f32, tag="ph2", bufs=4,
                                  name=f"ph2_{g}_{cb}")
                    for kc in range(KCH):
                        nc.tensor.matmul(
                            ph2[:],
                            w2m[:, kc * H2 + cb * 128:kc * H2 + (cb + 1) * 128],
                            x1T[:, kc * NJ2:(kc + 1) * NJ2],
                            start=(kc == 0), stop=(kc == KCH - 1))
                    ht = wpool.tile([128, NJ2], bf16, tag="h2T",
                                    name=f"h2T_{g}_{cb}", bufs=6)
                    nc.scalar.activation(ht[:], ph2[:], AF.Gelu,
                                         bias=b2T[:, cb:cb + 1])
                    h2T.append(ht)

                h2T_prev = h2T
            pl3_prev = emit_l3(IPC // 2 - 1, h2T_prev)
            emit_collect(IPC // 2 - 1, pl3_prev)

            # ---------- sigmoid + store ----------
            osg = cpool.tile([IPC, N2], f32, tag="osg")
            nc.scalar.activation(osg[:], outst[:], AF.Sigmoid, bias=b3t[:])
            nc.sync.dma_start(out=out_c[:], in_=osg[:])

    nc.finalize()
    return nc


def kernel(**inputs):
    from concourse.bass_utils import run_bass_kernel_spmd

    global _CACHED_NC
    f1 = np.ascontiguousarray(np.asarray(inputs["f1"], dtype=np.float32))
    f2 = np.ascontiguousarray(np.asarray(inputs["f2"], dtype=np.float32))
    W1 = np.ascontiguousarray(np.asarray(inputs["W1"], dtype=np.float32))
    b1 = np.asarray(inputs["b1"], dtype=np.float32)
    W2 = np.ascontiguousarray(np.asarray(inputs["W2"], dtype=np.float32))
    b2 = np.asarray(inputs["b2"], dtype=np.float32)
    W3 = np.ascontiguousarray(np.asarray(inputs["W3"], dtype=np.float32))
    b3 = np.asarray(inputs["b3"], dtype=np.float32)
    b3r = np.full((IPC, 1), b3.reshape(-1)[0], dtype=np.float32)

    if _CACHED_NC is None:
        _CACHED_NC = build_kernel()
    nc = _CACHED_NC

    in_maps = []
    for k in range(N_CORES):
        in_maps.append({
            "f1c": np.ascontiguousarray(f1[k * IPC:(k + 1) * IPC]),
            "f2c": np.ascontiguousarray(f2[k * JPC:(k + 1) * JPC]),
            "W1": W1, "b1": b1, "W2": W2, "b2": b2, "W3": W3, "b3r": b3r,
        })
    res = run_bass_kernel_spmd(nc, in_maps, core_ids=list(range(N_CORES)))
    out = np.concatenate([res.results[k]["out_c"] for k in range(N_CORES)],
                         axis=0)
    return out.astype(np.float32)
